# revision 1
# baseline (speedup 1.0000x reference)
"""Bahdanau attention weights kernel for 8 Trainium2 NeuronCores.

Reference computation (per full input):
    proj_enc = encoder_output @ W1_w + W1_b            # [B,S,U]
    proj_h   = last_layer_h_n @ W2_w + W2_b            # [B,1,U]
    score    = tanh(proj_enc + proj_h) @ V_w + V_b     # [B,S,1]
    out      = softmax(score, axis=1)                  # [B,S,1]

Sharding: data-parallel over batch. Each of the 8 cores gets B/8 batches;
weights are replicated; softmax is over the local sequence axis, so no
cross-core communication is needed.

Per-core layout strategy: keep U on partitions.
  - W1 [h,u] is the matmul stationary operand in its natural layout.
  - X^T tiles ([h, t]) DMA directly from the host-transposed bf16
    encoder copy (the f32r fallback builds them with PE transposes).
  - Main matmuls compute proj^T [u=128, t=512] in PSUM, accumulated over
    8 h-blocks, in a low-precision compute dtype LP (bf16 by default;
    float32r keeps near-fp32 accuracy at the same PE rate).
  - tanh runs on the scalar engine reading PSUM, with the combined bias
    (W1_b + W2_b + h_n @ W2)[u] as the per-partition bias operand.
  - The V contraction runs on the DVE: acc += V_ub (.) tanh_ub with V as
    a per-partition f32 scalar, then one all-ones matmul per group sums
    the accumulator over its 128 partitions.
  - Softmax per batch is Exp(accum_out) / reciprocal / tensor_scalar on
    the [1, 2048] score row (scores are bounded, so no max subtraction).
"""

import sys

for _p in ("/opt/trn_rl_repo", "/root/.axon_site/_ro/trn_rl_repo"):
    if _p not in sys.path:
        sys.path.append(_p)

import numpy as np

import concourse.bacc as bacc
import concourse.tile as tile
from concourse import mybir
from concourse.masks import make_identity

F32 = mybir.dt.float32
F32R = mybir.dt.float32r
BF16 = mybir.dt.bfloat16

B, S, H, U = 32, 2048, 1024, 1024
N_CORES = 8
B_LOCAL = B // N_CORES  # 4
P = 128
T_GROUP = 512  # tokens per group (matmul moving dim)


def build_kernel(b_local=B_LOCAL, s=S, h=H, u=U, x_bf16=True):
    """Build the per-core Bass program. Shape params must keep:
    s % T_GROUP == 0, h % 128 == 0, u % 512 == 0, u/128 divisible by 4.

    In the bf16 configuration the large inputs (encoder_output, W1_w,
    W2_w, V_w, last_layer_h_n) are expected PRE-CONVERTED to bf16 on the
    host: identical rounding to an on-chip cast, but half the DMA bytes
    and no cast work on the engines."""
    nc = bacc.Bacc()

    LP = BF16 if x_bf16 else F32R
    n_tok = b_local * s
    n_groups = n_tok // T_GROUP
    groups_per_batch = s // T_GROUP
    HB = h // P   # h blocks
    UB = u // P   # u blocks
    UH = u // T_GROUP  # 512-wide u halves (for the bias matmul)
    TSUB = T_GROUP // P
    QUAD = min(4, UB)  # V-matmuls packed per PSUM column-group set
    assert UB % QUAD == 0

    IDT = LP if x_bf16 else F32
    if x_bf16:
        # host supplies encoder_output and last_layer_h_n TRANSPOSED
        # ([h, tokens] / [h, b]) so X^T tiles DMA straight into SBUF
        enc = nc.dram_tensor("encoder_output", [h, n_tok], IDT,
                             kind="ExternalInput")
        hn = nc.dram_tensor("last_layer_h_n", [h, b_local], IDT,
                            kind="ExternalInput")
    else:
        enc = nc.dram_tensor("encoder_output", [n_tok, h], IDT,
                             kind="ExternalInput")
        hn = nc.dram_tensor("last_layer_h_n", [b_local, h], IDT,
                            kind="ExternalInput")
    w1 = nc.dram_tensor("W1_w", [h, u], IDT, kind="ExternalInput")
    b1 = nc.dram_tensor("W1_b", [u], F32, kind="ExternalInput")
    w2 = nc.dram_tensor("W2_w", [h, u], IDT, kind="ExternalInput")
    b2 = nc.dram_tensor("W2_b", [u], F32, kind="ExternalInput")
    vw = nc.dram_tensor("V_w", [u, 1], F32, kind="ExternalInput")
    vb = nc.dram_tensor("V_b", [1], F32, kind="ExternalInput")
    out = nc.dram_tensor("out", [b_local, s], F32, kind="ExternalOutput")

    if x_bf16:
        encT_v = enc.ap().rearrange("(hb p) (g t) -> g p hb t", p=P, t=T_GROUP)
        hnT_v = hn.ap().rearrange("(hb p) b -> p hb b", p=P)
    else:
        enc_v = enc.ap().rearrange("(g i p) h -> g i p h", i=TSUB, p=P)
    w1_v = w1.ap().rearrange("(hb p) u -> hb p u", p=P)
    w2_v = w2.ap().rearrange("(hb p) u -> hb p u", p=P)

    NPREF = 5 if x_bf16 else 2
    XBUFS = (NPREF + 2) * TSUB if x_bf16 else 2 * TSUB
    XTBUFS = NPREF + 1 if x_bf16 else 2

    with tile.TileContext(nc) as tc:
        with (
            tc.tile_pool(name="consts", bufs=1) as consts,
            tc.tile_pool(name="wpool", bufs=1) as wpool,
            tc.tile_pool(name="xpool", bufs=XBUFS) as xpool,
            tc.tile_pool(name="xtpool", bufs=XTBUFS) as xtpool,
            tc.tile_pool(name="thpool", bufs=3) as thpool,
            tc.tile_pool(name="scpool", bufs=2) as scpool,
            tc.tile_pool(name="smpool", bufs=2) as smpool,
            tc.tile_pool(name="pst", bufs=2, space="PSUM") as pst,
            tc.tile_pool(name="psu", bufs=2, space="PSUM") as psu,
            tc.tile_pool(name="pssc", bufs=2, space="PSUM") as pssc,
            tc.tile_pool(name="psmg", bufs=2, space="PSUM") as psmg,
        ):
            # ---- constants -------------------------------------------------
            ident = consts.tile([P, P], F32)
            make_identity(nc, ident)
            identL = consts.tile([P, P], LP)
            nc.vector.tensor_copy(identL, ident)

            # PE clock warm-up: ~3.5us of dummy matmuls on the identity run
            # inside the initial DMA window, so the HAM un-throttles the PE
            # before the first real matmul (cold rate is half speed)
            if x_bf16:
                warm_ps = pssc.tile([P, T_GROUP], F32, tag="warm")
                for _ in range(30):
                    nc.tensor.matmul(warm_ps[:, :P], lhsT=identL, rhs=identL)

            # prefetch the first groups' X tiles ahead of the weight DMAs so
            # the PE has transpose work during the weight-load phase
            PREFETCH = NPREF
            x_pending = {}

            def issue_x(g):
                if x_bf16:
                    xT = xtpool.tile([P, HB, T_GROUP], LP, tag="xT")
                    nc.sync.dma_start(out=xT, in_=encT_v[g])
                    x_pending[g] = xT
                    return
                tiles = []
                for i in range(TSUB):
                    xt = xpool.tile([P, h], F32, tag="x")
                    nc.sync.dma_start(out=xt, in_=enc_v[g, i])
                    xL = xpool.tile([P, h], LP, tag="x16")
                    nc.vector.tensor_copy(xL, xt)
                    tiles.append(xL)
                x_pending[g] = tiles

            # V in [u_p, u_blk] layout, f32 (only the DVE reads it as a
            # per-partition scalar, which must be f32)
            v_sb = consts.tile([P, UB], F32)
            nc.sync.dma_start(
                out=v_sb, in_=vw.ap().rearrange("(ub p) one -> p (ub one)", p=P)
            )
            vb_sb = consts.tile([1, 1], F32)
            nc.sync.dma_start(out=vb_sb, in_=vb.ap().rearrange("(a b) -> a b", a=1))

            # all-ones column: one matmul sums the V-weighted tanh
            # accumulator over its 128 partitions
            ones_sb = consts.tile([P, 1], LP)
            nc.vector.memset(ones_sb, 1.0)

            # W1_b + W2_b in [u_p, u_blk] layout
            b1_sb = consts.tile([P, UB], F32)
            nc.sync.dma_start(out=b1_sb, in_=b1.ap().rearrange("(ub p) -> p ub", p=P))
            b2_sb = consts.tile([P, UB], F32)
            nc.sync.dma_start(out=b2_sb, in_=b2.ap().rearrange("(ub p) -> p ub", p=P))
            b12_sb = consts.tile([P, UB], F32)
            nc.vector.tensor_add(b12_sb, b1_sb, b2_sb)

            # h_n^T [h=128, hb, b] (host-transposed in the bf16 path)
            if x_bf16:
                hnT = consts.tile([P, HB, b_local], LP)
                nc.sync.dma_start(out=hnT, in_=hnT_v)
            else:
                hn_f32 = consts.tile([b_local, h], F32)
                nc.sync.dma_start(out=hn_f32, in_=hn.ap())
                hn_sb = consts.tile([b_local, h], LP)
                nc.vector.tensor_copy(hn_sb, hn_f32)

            # Weights: W2 first (it gates the bias chain, the PE's first
            # real work), then X(0) and W1 (which gate the main matmuls),
            # then the rest of the X prefetch.
            w1_sb = []
            w2_sb = []
            if x_bf16:
                for hb in range(HB):
                    t2 = wpool.tile([P, u], LP, tag=f"w2b_{hb}")
                    nc.sync.dma_start(out=t2, in_=w2_v[hb])
                    w2_sb.append(t2)
                issue_x(0)
                for hb in range(HB):
                    t1 = wpool.tile([P, u], LP, tag=f"w1b_{hb}")
                    nc.sync.dma_start(out=t1, in_=w1_v[hb])
                    w1_sb.append(t1)
                for g0 in range(1, min(PREFETCH, n_groups)):
                    issue_x(g0)
            else:
                issue_x(0)
                with tc.tile_pool(name="wstage", bufs=2) as wstage:
                    for hb in range(HB):
                        stg2 = xpool.tile([P, u], F32, tag="x")
                        nc.sync.dma_start(out=stg2, in_=w2_v[hb])
                        t2 = wpool.tile([P, u], LP, tag=f"w2b_{hb}")
                        nc.vector.tensor_copy(t2, stg2)
                        w2_sb.append(t2)
                        stg1 = wstage.tile([P, u], F32, tag="w1s")
                        nc.sync.dma_start(out=stg1, in_=w1_v[hb])
                        t1 = wpool.tile([P, u], LP, tag=f"w1b_{hb}")
                        nc.vector.tensor_copy(t1, stg1)
                        w1_sb.append(t1)
                for g0 in range(1, min(PREFETCH, n_groups)):
                    issue_x(g0)

            if not x_bf16:
                # transpose h_n -> hnT [h=128, b] blocks (LP)
                hnT = consts.tile([P, HB, b_local], LP)
                for hb in range(HB):
                    ps = pst.tile([P, T_GROUP], LP, tag="tp")
                    nc.tensor.transpose(
                        ps[:, :b_local], hn_sb[:, hb * P : (hb + 1) * P],
                        identL[:b_local, :b_local],
                    )
                    nc.vector.tensor_copy(hnT[:, hb, :], ps[:, :b_local])

            # ---- bias precompute: bias[u, b] = h_n @ W2 + (b1 + b2) --------
            # computed as [b, u] with W2 as the 512-wide moving operand,
            # then transposed back to [u, b] blocks
            bias_sb = consts.tile([P, UB, b_local], F32)
            for uh in range(UH):
                ps4 = pst.tile([P, T_GROUP], F32, tag="tp")
                for hb in range(HB):
                    nc.tensor.matmul(
                        ps4[:b_local, :],
                        lhsT=hnT[:, hb, :],
                        rhs=w2_sb[hb][:, uh * T_GROUP : (uh + 1) * T_GROUP],
                        start=(hb == 0),
                        stop=(hb == HB - 1),
                    )
                bstage = thpool.tile([b_local, T_GROUP], F32, tag="bstage")
                nc.vector.tensor_copy(bstage, ps4[:b_local, :])
                for i in range(TSUB):
                    ub = uh * TSUB + i
                    psb_t = pst.tile([P, T_GROUP], F32, tag="tp")
                    nc.tensor.transpose(
                        psb_t[:, :b_local],
                        bstage[:, i * P : (i + 1) * P],
                        ident[:b_local, :b_local],
                    )
                    nc.scalar.activation(
                        bias_sb[:, ub, :], psb_t[:, :b_local],
                        mybir.ActivationFunctionType.Identity,
                        bias=b12_sb[:, ub : ub + 1],
                    )

            # ---- main loop over token groups ------------------------------
            # The merge/exp/normalize of group g-1 is emitted after group
            # g's transposes so the PE never waits on the small DVE copy
            # that feeds the merge matmul.
            state = {"sc_row": None, "esums": None, "pending": None}

            def finish_dve(acc):
                scm = thpool.tile([P, T_GROUP], LP, tag="scm")
                nc.vector.tensor_copy(scm, acc)
                return scm

            def finish_pe(scm, pb, pgi):
                score_ps = psmg.tile([1, T_GROUP], F32, tag="mg")
                nc.tensor.matmul(score_ps, lhsT=ones_sb, rhs=scm)
                # score chunk -> exp incrementally per chunk (adds V_b).
                # scores are bounded (|score| <= sum|V_w|+|V_b| < 17), so
                # exp without max-subtraction is safe in fp32.
                if pgi == 0:
                    state["sc_row"] = scpool.tile(
                        [1, s], F32, tag="scrow", name="sc_row")
                    state["esums"] = smpool.tile(
                        [1, groups_per_batch], F32, tag="esums", name="esums")
                sc_row, esums = state["sc_row"], state["esums"]
                nc.scalar.activation(
                    sc_row[:, pgi * T_GROUP : (pgi + 1) * T_GROUP], score_ps,
                    mybir.ActivationFunctionType.Exp,
                    bias=vb_sb,
                    accum_out=esums[:, pgi : pgi + 1],
                )
                if pgi == groups_per_batch - 1:
                    esum = smpool.tile([1, 1], F32, tag="esum")
                    nc.vector.tensor_reduce(
                        esum, esums, axis=mybir.AxisListType.X,
                        op=mybir.AluOpType.add,
                    )
                    rec = smpool.tile([1, 1], F32, tag="rec")
                    nc.vector.reciprocal(rec, esum)
                    nc.vector.tensor_scalar_mul(sc_row, sc_row, rec)
                    nc.sync.dma_start(out=out.ap()[pb : pb + 1, :], in_=sc_row)

            for g in range(n_groups):
                b = g // groups_per_batch
                gi = g % groups_per_batch

                if g + PREFETCH < n_groups:
                    issue_x(g + PREFETCH)

                if state["pending"] is not None:
                    psq, pb, pgi = state["pending"]
                    scm_prev = finish_dve(psq)
                else:
                    scm_prev = None

                if x_bf16:
                    # X^T arrives transposed straight from DRAM
                    xT = x_pending.pop(g)
                else:
                    xL_tiles = x_pending.pop(g)
                    # transpose to X^T [h=128, t=512] blocks on the PE
                    xT = xtpool.tile([P, HB, T_GROUP], LP, tag="xT")
                    for hb in range(HB):
                        ps = pst.tile([P, T_GROUP], LP, tag="tp")
                        for i in range(TSUB):
                            nc.tensor.transpose(
                                ps[:, i * P : (i + 1) * P],
                                xL_tiles[i][:, hb * P : (hb + 1) * P],
                                identL,
                            )
                        nc.vector.tensor_copy(xT[:, hb, :], ps)

                # proj^T[u, t] blocks + tanh; the V contraction runs on
                # the DVE as acc += V_ub (.) tanh_ub (per-partition scalar),
                # leaving the PE only one ones-matmul per group
                acc = scpool.tile([P, T_GROUP], F32, tag="acc", bufs=3)
                for ub in range(UB):
                    pu = psu.tile([P, T_GROUP], F32, tag="pu")
                    for hb in range(HB):
                        nc.tensor.matmul(
                            pu,
                            lhsT=w1_sb[hb][:, ub * P : (ub + 1) * P],
                            rhs=xT[:, hb, :],
                            start=(hb == 0),
                            stop=(hb == HB - 1),
                        )
                    th = thpool.tile([P, T_GROUP], LP, tag="th", bufs=4)
                    nc.scalar.activation(
                        th, pu,
                        mybir.ActivationFunctionType.Tanh,
                        bias=bias_sb[:, ub, b : b + 1],
                    )
                    if ub == 0:
                        nc.vector.tensor_scalar_mul(
                            acc, th, v_sb[:, 0:1])
                    else:
                        nc.vector.scalar_tensor_tensor(
                            acc, th, v_sb[:, ub : ub + 1], acc,
                            op0=mybir.AluOpType.mult,
                            op1=mybir.AluOpType.add,
                        )
                    if ub == 0 and scm_prev is not None:
                        # merge of the previous group lands here, after a
                        # full matmul chain has hidden its DVE copy
                        finish_pe(scm_prev, pb, pgi)
                        scm_prev = None
                        state["pending"] = None
                state["pending"] = (acc, b, gi)

            # flush the last group
            psq, pb, pgi = state["pending"]
            finish_pe(finish_dve(psq), pb, pgi)

    nc.compile()
    return nc


def make_in_maps(inputs, x_bf16=True):
    """Shard the full inputs per core. In the bf16 configuration the big
    tensors are pre-rounded to bf16 and encoder_output / last_layer_h_n
    are pre-transposed to [H, tokens] / [H, b] on the host."""
    import ml_dtypes

    bf16 = ml_dtypes.bfloat16

    def f32(name):
        return np.ascontiguousarray(np.asarray(inputs[name], dtype=np.float32))

    def big(name):
        a = f32(name)
        return a.astype(bf16) if x_bf16 else a

    enc = big("encoder_output")
    hn = big("last_layer_h_n")
    w1, w2 = big("W1_w"), big("W2_w")
    vw = f32("V_w")
    b1, b2, vb = f32("W1_b"), f32("W2_b"), f32("V_b")

    in_maps = []
    for c in range(N_CORES):
        sl = slice(c * B_LOCAL, (c + 1) * B_LOCAL)
        e = enc[sl].reshape(B_LOCAL * S, H)
        n = hn[sl]
        if x_bf16:
            e = e.T  # [H, tokens]
            n = n.T  # [H, b]
        in_maps.append({
            "encoder_output": np.ascontiguousarray(e),
            "last_layer_h_n": np.ascontiguousarray(n),
            "W1_w": w1, "W1_b": b1, "W2_w": w2, "W2_b": b2,
            "V_w": vw, "V_b": vb,
        })
    return in_maps


def kernel(**inputs):
    from concourse.bass_utils import run_bass_kernel_spmd

    nc = build_kernel()
    in_maps = make_in_maps(inputs)
    res = run_bass_kernel_spmd(nc, in_maps, core_ids=list(range(N_CORES)))
    outs = [res.results[c]["out"].reshape(B_LOCAL, S, 1) for c in range(N_CORES)]
    return np.concatenate(outs, axis=0)

